# revision 27
# baseline (speedup 1.0000x reference)
"""Trainium2 Bass kernel for nn_CaserQueryEncoder.

Model (B=1024, L=50, D=128, NV=8, NH=16):
  P_u = user_emb[user_ids]                                   [B, D]
  E   = item_emb[item_seq]                                   [B, L, D]
  o_v = einsum('btd,vt->bvd', E, Wv) + bv                    [B, NV*D]
  conv[b,i,j,t] = sum_{dt<=i} <E[b, t+dt, :], Wh[i,j,dt,:]>  (Wh zero for dt>i)
  o_h[b,i,j] = max over valid t (t <= 49-i) of relu(conv + bh)
  z = relu([o_v, o_h] @ fc_W + fc_b)                         [B, D]
  out = [z, P_u]                                             [B, 2D]

Strategy: pure data parallel, 128 batch rows per core x 8 cores, no
collectives. Per core:
  - item embeddings stored bf16 in DRAM; looked-up rows arrive via batched
    indirect DMAs (7 groups of ~8 seq positions -> ~1k descriptors each,
    amortizing the ~1us fixed SWDGE cost that a per-position gather pays
    50x). PE transposes (bf16, via identity) build ET[d, b, t'].
  - horizontal conv in bf16 as PSUM-accumulated shifted matmuls: heights
    packed 8 per chunk (x16 filters = 128 weight columns); for each tap dt
    the rhs is ET shifted by dt; PSUM accumulates over taps. Per-tap
    t-windows are trimmed to min(nt, 50-dt) (later columns only read the
    zero pad). Invalid (height, t) cells get an additive -1e30 mask before
    the max-reduce; max(relu(x+b)) == relu(max(x)+b) so relu+bias follow
    the reduce.
  - blocks are processed in small sets with taps outermost so consecutive
    matmuls share stationary weights; those continuation matmuls set
    ldweights=False to skip the per-matmul weight reload.
  - vertical conv never materialized: G[t,d,k] = sum_v Wv[v,t]*fc_W[v*128+d,k]
    is precomputed on host (bf16) and ET_t @ G_t is accumulated straight
    into the fc PSUM.
  - fc bias added via a K=1 ones-matmul; relu on the scalar engine.
"""

import math
import os
import sys
from contextlib import ExitStack

import numpy as np
import ml_dtypes

sys.path.insert(0, "/opt/trn_rl_repo")

import concourse.bass as bass
import concourse.tile as tile
from concourse import mybir
from concourse.bass import IndirectOffsetOnAxis
from concourse.bass_utils import run_bass_kernel_spmd
from concourse.masks import make_identity
from concourse.vector_clock import ScopedClock


def _patch_tile_drain():
    """This container's walrus codegen only accepts one sync-wait per Drain
    (CTRL_NO_STRUCT); Tile's kernel-tail drain carries one wait per live
    semaphore. Split the waits across a chain of drains, one wait each."""
    if getattr(tile.TileContext, "_drain_split_patched", False):
        return

    def _patched(self, tick_clock, wait_clock):
        nc = self.nc
        probe = nc.sync.drain()
        wait_clock.add_sem_waits(
            probe.ins, ScopedClock({None: tick_clock.global_clock}))
        nc.all_engine_barrier()
        popped = nc._tile_sem_poison_stack.pop()
        assert popped is self._sem_poison
        # all engines are quiesced by the barrier above; the sem clear runs
        # on one engine and nothing executes after it, so the second
        # barrier the stock drain emits (~3-4us of split sem-waits under
        # this container's one-wait-per-instruction codegen) is dropped.
        nc.clear_and_free_semaphores(list(self.sems.allocated().values()))

    tile.TileContext._drain_and_barrier = _patched
    tile.TileContext._drain_split_patched = True


_patch_tile_drain()


def _split_json_waits(j, max_waits=1):
    """This walrus codegen accepts at most one sync-wait per instruction.
    Hoist extra waits onto wait-only EventSemaphore instructions inserted
    just before the offender on the same engine queue."""
    n = 0
    for fn in j["functions"]:
        for blk in fn["blocks"]:
            out = []
            for inst in blk["instructions"]:
                si = inst.get("sync_info")
                waits = (si or {}).get("on_wait") or []
                if len(waits) > max_waits:
                    for k, w in enumerate(waits[:-max_waits]):
                        out.append({
                            "debug": inst.get("debug", 0),
                            "engine": inst["engine"],
                            "ins": [], "outs": [],
                            "name": f"{inst['name']}_wsplit{k}",
                            "opcode": "EventSemaphore",
                            "sync_info": {"on_update": [], "on_wait": [w]},
                        })
                        n += 1
                    si["on_wait"] = waits[-max_waits:]
                out.append(inst)
            blk["instructions"] = out
    return n


def _install_wait_splitter(nc):
    import json as _json

    orig = nc.to_json_bytes

    def patched():
        j = _json.loads(orig())
        _split_json_waits(j)
        return _json.dumps(j).encode()

    nc.to_json_bytes = patched

B = 1024
L = 50
D = 128
NV = 8
NH = 16
NU = 100000
NI = 500000
NCORES = 8
BLOC = B // NCORES          # 128 batch rows per core
TP = 64                     # t' pitch in ET
NEG = -1.0e30
FC_IN = NV * D + NH * L     # 1824
NOUT = 2 * D                # 256

LDW_REUSE = bool(int(os.environ.get("BASS_LDW_REUSE", "0")))

# Height-chunk table: heights [8u, 8u+nh) packed as m2 = 16*(i-8u)+j.
# ndt taps accumulate in PSUM; Nt is the t-window (valid-t of the chunk's
# shortest filter); Nb batch rows per matmul so that Nb*Nt <= 512.
CHUNKS = []
_base = 0
for _u in range(7):
    _i0 = 8 * _u
    _nh = min(8, L - _i0)
    _ndt = min(_i0 + 8, L)
    _nt = L - _i0
    _nb = min(BLOC, 512 // _nt)
    _nblk = math.ceil(BLOC / _nb)
    CHUNKS.append(dict(i0=_i0, nh=_nh, ndt=_ndt, nt=_nt, nb=_nb,
                       nblk=_nblk, base=_base))
    _base += _ndt
NWTILES = _base             # 218 weight tiles of [d=128, m2=128]

# conv matmuls for these (u, blk) chase the gather stream; each uses one
# cpsum bank for the whole gather window.
PHASE_A = [(6, 0), (5, 0), (5, 1), (5, 2), (4, 0), (4, 1)]

_NC_CACHE = None

# Set BASS_KERNEL_TRACE=1 to profile; exec time lands in LAST_RESULTS.
LAST_RESULTS = None


def _build_nc():
    f32 = mybir.dt.float32
    bf16 = mybir.dt.bfloat16
    i32 = mybir.dt.int32
    X = mybir.AxisListType.X

    nc = bass.Bass()
    seq_t = nc.dram_tensor("seq_idx", [BLOC, L], i32, kind="ExternalInput")
    uid_t = nc.dram_tensor("uid_idx", [BLOC, 1], i32, kind="ExternalInput")
    item_t = nc.dram_tensor("item_emb", [NI, D], bf16, kind="ExternalInput")
    user_t = nc.dram_tensor("user_emb", [NU, D], f32, kind="ExternalInput")
    whp_t = nc.dram_tensor("whp", [D, NWTILES * 128], bf16, kind="ExternalInput")
    g_t = nc.dram_tensor("g", [D, L * D], bf16, kind="ExternalInput")
    fcwh_t = nc.dram_tensor("fcwh", [128, 7 * D], bf16, kind="ExternalInput")
    masks_t = nc.dram_tensor("masks", [128, 7 * 512], bf16, kind="ExternalInput")
    bh_t = nc.dram_tensor("bh_p", [128, 7], f32, kind="ExternalInput")
    fcb_t = nc.dram_tensor("fcb", [1, D], f32, kind="ExternalInput")
    out_t = nc.dram_tensor("out", [BLOC, NOUT], f32, kind="ExternalOutput")

    with ExitStack() as ctx:
        tc = ctx.enter_context(tile.TileContext(nc))
        const = ctx.enter_context(tc.tile_pool(name="const", bufs=1))
        egath = ctx.enter_context(tc.tile_pool(name="egath", bufs=8))
        etp = ctx.enter_context(tc.tile_pool(name="etp", bufs=1))
        wpool = ctx.enter_context(tc.tile_pool(name="wpool", bufs=1))
        ohp = ctx.enter_context(tc.tile_pool(name="ohp", bufs=1))
        misc = ctx.enter_context(tc.tile_pool(name="misc", bufs=1))
        tpsum = ctx.enter_context(tc.tile_pool(name="tpsum", bufs=1, space="PSUM"))
        cpsum = ctx.enter_context(tc.tile_pool(name="cpsum", bufs=6, space="PSUM"))
        zpsum = ctx.enter_context(tc.tile_pool(name="zpsum", bufs=1, space="PSUM"))

        # --- small constants; seq first (it gates the gathers) ---
        seq_sb = const.tile([BLOC, L], i32)
        nc.sync.dma_start(out=seq_sb[:], in_=seq_t[:])
        uid_sb = const.tile([BLOC, 1], i32)
        nc.sync.dma_start(out=uid_sb[:], in_=uid_t[:])

        # identity must precede the gathers on the gpsimd queue (transposes
        # need it almost immediately; the gather stream occupies the queue
        # for ~55us)
        ident = const.tile([128, 128], bf16)
        make_identity(nc, ident[:])

        # --- per-position indirect gathers (multi-offset DGE is broken on
        # this hw; k=1 is the proven shape). ~1.1us of serial SWDGE each,
        # hidden behind the conv chase. ---
        eg_tiles = []
        for t in range(L):
            eg = egath.tile([BLOC, D], bf16, tag="eg", name=f"eg{t}")
            nc.gpsimd.indirect_dma_start(
                out=eg[:], out_offset=None, in_=item_t[:],
                in_offset=IndirectOffsetOnAxis(ap=seq_sb[:, t:t + 1], axis=0))
            eg_tiles.append(eg)
        pu_sb = misc.tile([BLOC, D], f32, tag="pu")
        nc.gpsimd.indirect_dma_start(
            out=pu_sb[:], out_offset=None, in_=user_t[:],
            in_offset=IndirectOffsetOnAxis(ap=uid_sb[:, :1], axis=0))

        fcb_sb = const.tile([1, D], f32)
        nc.sync.dma_start(out=fcb_sb[:], in_=fcb_t[:])
        bh_sb = const.tile([128, 7], f32)
        nc.sync.dma_start(out=bh_sb[:], in_=bh_t[:])
        ones_sb = const.tile([1, BLOC], f32)
        nc.vector.memset(ones_sb[:], 1.0)

        # --- weight prefetch: everything issued up front, split into
        # pieces, spread over the sync + scalar queues in need order so no
        # conv phase ever waits on a weight transfer ---
        wu_sb = {}
        for u, ch in enumerate(CHUNKS):
            wu_sb[u] = wpool.tile([D, ch["ndt"] * 128], bf16, tag=f"w{u}",
                                  name=f"wu{u}")
        g_sb = const.tile([D, L * D], bf16)
        mask_sb = const.tile([128, 7 * 512], bf16)
        fcwh_sb = const.tile([128, 7 * D], bf16)

        def wu_piece(eng, u, d0, d1):
            ch = CHUNKS[u]
            d1 = min(d1, ch["ndt"])
            eng.dma_start(
                out=wu_sb[u][:, d0 * 128:d1 * 128],
                in_=whp_t[:, (ch["base"] + d0) * 128:(ch["base"] + d1) * 128])

        # need order: u6/u5 taps stream with the gathers; u4 from ~15us;
        # masks at ~55us; then u3/u2/u1/u0 paced by the main phase.
        wu_piece(nc.sync, 6, 0, 16)
        wu_piece(nc.scalar, 5, 0, 16)
        wu_piece(nc.sync, 6, 16, 50)
        wu_piece(nc.scalar, 5, 16, 48)
        nc.sync.dma_start(out=g_sb[:, :25 * D], in_=g_t[:, :25 * D])
        nc.scalar.dma_start(out=g_sb[:, 25 * D:], in_=g_t[:, 25 * D:])
        wu_piece(nc.sync, 4, 0, 20)
        wu_piece(nc.scalar, 4, 20, 40)
        nc.sync.dma_start(out=mask_sb[:], in_=masks_t[:])
        nc.scalar.dma_start(out=fcwh_sb[:], in_=fcwh_t[:])
        wu_piece(nc.sync, 3, 0, 16)
        wu_piece(nc.scalar, 3, 16, 32)
        wu_piece(nc.sync, 2, 0, 24)
        wu_piece(nc.scalar, 1, 0, 16)
        wu_piece(nc.sync, 0, 0, 8)

        # --- ET[d, t', b] in bf16: t-major with b contiguous, so conv
        # matmuls stream unit-stride columns in t-major psum order ---
        et = etp.tile([D, TP, BLOC], bf16)

        # --- fc accumulation PSUM [b, k]; opened by the bias matmul inside
        # the chase (after the first transposes), closed by the last fc
        # matmul. ---
        zp = zpsum.tile([BLOC, D], f32)

        # 4 transpose slots in one PSUM bank; pair-copies drain them.
        tp = tpsum.tile([128, 4, 128], bf16)

        # --- conv bookkeeping ---
        psum_tiles = {}
        fc_pending = []
        oh_tiles = {}
        blocks_left = [ch["nblk"] for ch in CHUNKS]
        n_fc_left = [7]

        def get_ohu(u):
            if u not in oh_tiles:
                oh_tiles[u] = ohp.tile([128, BLOC], bf16, tag=f"oh{u}",
                                       name=f"oh{u}")
            return oh_tiles[u]

        def flush_fc(all_=False):
            keep = 0
            while len(fc_pending) > keep:
                uu = fc_pending.pop(0)
                n_fc_left[0] -= 1
                nc.tensor.matmul(out=zp[:], lhsT=oh_tiles[uu][:],
                                 rhs=fcwh_sb[:, uu * D:(uu + 1) * D],
                                 start=False, stop=(n_fc_left[0] == 0))

        def emit_conv(u, blk, dt, reuse, red_eng=None):
            # conv PSUM layout is t-major (col = t*nbb + b) so a trimmed
            # tap's window is a contiguous 2D prefix of the bank.
            ch = CHUNKS[u]
            nt, nb, ndt = ch["nt"], ch["nb"], ch["ndt"]
            b0 = blk * nb
            nbb = min(nb, BLOC - b0)
            n = nbb * nt
            w = min(nt, L - dt)
            key = (u, blk)
            if key not in psum_tiles:
                psum_tiles[key] = cpsum.tile([128, 512], f32, tag="cps",
                                             name=f"cps_{u}_{blk}")
            ps = psum_tiles[key]
            rhs = et[:, dt:dt + w, b0:b0 + nbb]
            mm = nc.tensor.matmul(
                out=ps[:, :nbb * w],
                lhsT=wu_sb[u][:, dt * 128:(dt + 1) * 128],
                rhs=rhs,
                start=(dt == 0), stop=(dt == ndt - 1),
                skip_group_check=True)
            if reuse and LDW_REUSE:
                mm.ins.ldweights = False
            if dt == ndt - 1:
                # only heights r>0 of the chunk have invalid trailing t
                # positions (at most the last 7 columns) -> mask just those
                ps3 = ps[:, :n].rearrange("p (t b) -> p t b", b=nbb)
                m0 = max(0, nt - 7)
                pst = ps[:, m0 * nbb:n].rearrange("p (t b) -> p t b", b=nbb)
                m3 = mask_sb[:, u * 512 + m0 * nb:u * 512 + nt * nb].rearrange(
                    "p (t b) -> p t b", b=nb)[:, :, :nbb]
                nc.vector.tensor_tensor(
                    out=pst, in0=pst, in1=m3, op=mybir.AluOpType.add)
                nc.vector.reduce_max(
                    out=get_ohu(u)[:, b0:b0 + nbb],
                    in_=ps3.rearrange("p t b -> p b t"),
                    axis=X)
                del psum_tiles[key]
                blocks_left[u] -= 1
                if blocks_left[u] == 0:
                    ohu = get_ohu(u)
                    nc.scalar.activation(ohu[:], ohu[:],
                                         mybir.ActivationFunctionType.Relu,
                                         bias=bh_sb[:, u:u + 1])
                    fc_pending.append(u)

        # chase taps: (u, dt) ready once ET cols [dt, dt+w) are copied;
        # copies land in pairs at odd t, so key on the covering odd column.
        chase = {}
        chase_us = sorted({u for (u, _) in PHASE_A})
        for u in chase_us:
            nt = CHUNKS[u]["nt"]
            for dt in range(CHUNKS[u]["ndt"]):
                w = min(nt, L - dt)
                c = dt + w - 1
                c += (c + 1) % 2
                chase.setdefault(min(c, L - 1), []).append((u, dt))

        # --- gather-chase: per position, transpose -> (odd t) pair copy.
        # The G matmuls + conv taps of a pair are deferred one pair so the
        # PE never idles waiting on the copy it just triggered. ---
        def emit_pair_work(c):
            if c < 1:
                return
            for tt in (c - 1, c):
                nc.tensor.matmul(out=zp[:], lhsT=et[:, tt, :],
                                 rhs=g_sb[:, tt * D:(tt + 1) * D],
                                 start=False, stop=False)
            for (u, dt) in chase.get(c, ()):
                blks = [blk for (uu, blk) in PHASE_A if uu == u]
                for j, blk in enumerate(blks):
                    emit_conv(u, blk, dt, reuse=(j > 0))

        for t in range(L):
            s = t % 4
            nc.tensor.transpose(out=tp[:, s, :], in_=eg_tiles[t][:],
                                identity=ident[:])
            if t % 2 == 0:
                continue
            nc.vector.tensor_copy(out=et[:, t - 1:t + 1, :],
                                  in_=tp[:, s - 1:s + 1, :])
            if t == 1:
                nc.tensor.matmul(out=zp[:], lhsT=ones_sb[:], rhs=fcb_sb[:],
                                 start=True, stop=False)
            emit_pair_work(t - 2)
        emit_pair_work(L - 1)

        # --- main phase: remaining blocks, taps outermost within small
        # sets so stationary weights are reused across the set ---
        done_a = set(PHASE_A)
        for u in [4, 3, 2, 1, 0]:
            ch = CHUNKS[u]
            rem = [blk for blk in range(ch["nblk"]) if (u, blk) not in done_a]
            sets = [rem[i:i + 3] for i in range(0, len(rem), 3)]
            for set3 in sets:
                flush_fc()
                for dt in range(ch["ndt"]):
                    for j, blk in enumerate(set3):
                        emit_conv(u, blk, dt, reuse=(j > 0))

        # --- remaining o_h fc matmuls close the zp group ---
        nc.sync.dma_start(out=out_t[:, D:NOUT], in_=pu_sb[:])
        flush_fc(all_=True)

        z_sb = misc.tile([BLOC, D], f32, tag="z")
        nc.scalar.activation(z_sb[:], zp[:], mybir.ActivationFunctionType.Relu)
        nc.sync.dma_start(out=out_t[:, 0:D], in_=z_sb[:])

    return nc


def _to_bf16(x):
    return np.ascontiguousarray(np.asarray(x, np.float32)).astype(
        ml_dtypes.bfloat16)


def _prep_common(user_emb, item_emb, Wv, bv, Wh, bh, fc_W, fc_b):
    f = np.float32
    item_emb = _to_bf16(item_emb)
    user_emb = np.ascontiguousarray(np.asarray(user_emb, f))
    Wh = np.asarray(Wh, f)          # [L, NH, L, D], zero for dt > i
    Wv = np.asarray(Wv, f)          # [NV, L]
    bv = np.asarray(bv, f)
    bh = np.asarray(bh, f)          # [L, NH]
    fc_W = np.asarray(fc_W, f)      # [FC_IN, D]
    fc_b = np.asarray(fc_b, f)

    whp = np.zeros((D, NWTILES * 128), f)
    masks = np.full((128, 7 * 512), 0.0, f)
    fcwh = np.zeros((128, 7 * D), f)
    bh_p = np.zeros((128, 7), f)
    fcw_h = fc_W[NV * D:]           # [800, D]
    for u, ch in enumerate(CHUNKS):
        i0, nh, ndt, nt, nb = ch["i0"], ch["nh"], ch["ndt"], ch["nt"], ch["nb"]
        base = ch["base"]
        wu = Wh[i0:i0 + nh]         # [nh, NH, L, D]
        for dt in range(ndt):
            blkw = wu[:, :, dt, :].reshape(nh * NH, D)
            whp[:, (base + dt) * 128:(base + dt) * 128 + nh * NH] = blkw.T
        m = np.full((128, nb * nt), NEG, f)
        for mm in range(nh * NH):
            i = i0 + mm // NH
            vt = min(L - i, nt)
            row = np.full((nt,), NEG, f)
            row[:vt] = 0.0
            m[mm] = np.repeat(row, nb)   # t-major: col = t*nb + b
        masks[:, u * 512:u * 512 + nb * nt] = m
        fcwh[:nh * NH, u * D:(u + 1) * D] = fcw_h[u * 128:u * 128 + nh * NH]
        bh_p[:nh * NH, u] = bh[i0:i0 + nh].reshape(nh * NH)

    fcv = fc_W[:NV * D].reshape(NV, D, D)
    g = np.einsum("vt,vdk->tdk", Wv, fcv)            # [L, D, D]
    g = np.ascontiguousarray(g.transpose(1, 0, 2).reshape(D, L * D))
    fcb = (fc_b + np.einsum("v,vdk->k", bv, fcv)).reshape(1, D).astype(f)

    return dict(item_emb=item_emb, user_emb=user_emb, whp=_to_bf16(whp),
                g=_to_bf16(g), fcwh=_to_bf16(fcwh), masks=_to_bf16(masks),
                bh_p=bh_p, fcb=fcb)


def make_in_maps(user_ids, item_seq, user_emb, item_emb, Wv, bv, Wh, bh,
                 fc_W, fc_b):
    common = _prep_common(user_emb, item_emb, Wv, bv, Wh, bh, fc_W, fc_b)
    user_ids = np.asarray(user_ids).astype(np.int32).reshape(B, 1)
    item_seq = np.asarray(item_seq).astype(np.int32).reshape(B, L)
    in_maps = []
    for c in range(NCORES):
        m = dict(common)
        m["seq_idx"] = np.ascontiguousarray(item_seq[c * BLOC:(c + 1) * BLOC])
        m["uid_idx"] = np.ascontiguousarray(user_ids[c * BLOC:(c + 1) * BLOC])
        in_maps.append(m)
    return in_maps


def get_nc():
    global _NC_CACHE
    if _NC_CACHE is None:
        _NC_CACHE = _build_nc()
        _install_wait_splitter(_NC_CACHE)
    return _NC_CACHE


def kernel(**inputs) -> np.ndarray:
    global LAST_RESULTS
    in_maps = make_in_maps(**inputs)
    nc = get_nc()
    trace = bool(int(os.environ.get("BASS_KERNEL_TRACE", "0")))
    res = run_bass_kernel_spmd(nc, in_maps, list(range(NCORES)), trace=trace)
    LAST_RESULTS = res
    return np.concatenate([res.results[c]["out"] for c in range(NCORES)], axis=0)


# revision 29
# speedup vs baseline: 1.0056x; 1.0056x over previous
"""Trainium2 Bass kernel for nn_CaserQueryEncoder.

Model (B=1024, L=50, D=128, NV=8, NH=16):
  P_u = user_emb[user_ids]                                   [B, D]
  E   = item_emb[item_seq]                                   [B, L, D]
  o_v = einsum('btd,vt->bvd', E, Wv) + bv                    [B, NV*D]
  conv[b,i,j,t] = sum_{dt<=i} <E[b, t+dt, :], Wh[i,j,dt,:]>  (Wh zero for dt>i)
  o_h[b,i,j] = max over valid t (t <= 49-i) of relu(conv + bh)
  z = relu([o_v, o_h] @ fc_W + fc_b)                         [B, D]
  out = [z, P_u]                                             [B, 2D]

Strategy: pure data parallel, 128 batch rows per core x 8 cores, no
collectives. Per core:
  - item embeddings stored bf16 in DRAM; looked-up rows arrive via batched
    indirect DMAs (7 groups of ~8 seq positions -> ~1k descriptors each,
    amortizing the ~1us fixed SWDGE cost that a per-position gather pays
    50x). PE transposes (bf16, via identity) build ET[d, b, t'].
  - horizontal conv in bf16 as PSUM-accumulated shifted matmuls: heights
    packed 8 per chunk (x16 filters = 128 weight columns); for each tap dt
    the rhs is ET shifted by dt; PSUM accumulates over taps. Per-tap
    t-windows are trimmed to min(nt, 50-dt) (later columns only read the
    zero pad). Invalid (height, t) cells get an additive -1e30 mask before
    the max-reduce; max(relu(x+b)) == relu(max(x)+b) so relu+bias follow
    the reduce.
  - blocks are processed in small sets with taps outermost so consecutive
    matmuls share stationary weights; those continuation matmuls set
    ldweights=False to skip the per-matmul weight reload.
  - vertical conv never materialized: G[t,d,k] = sum_v Wv[v,t]*fc_W[v*128+d,k]
    is precomputed on host (bf16) and ET_t @ G_t is accumulated straight
    into the fc PSUM.
  - fc bias added via a K=1 ones-matmul; relu on the scalar engine.
"""

import math
import os
import sys
from contextlib import ExitStack

import numpy as np
import ml_dtypes

sys.path.insert(0, "/opt/trn_rl_repo")

import concourse.bass as bass
import concourse.tile as tile
from concourse import mybir
from concourse.bass import IndirectOffsetOnAxis
from concourse.bass_utils import run_bass_kernel_spmd
from concourse.masks import make_identity
from concourse.vector_clock import ScopedClock


def _patch_tile_drain():
    """This container's walrus codegen only accepts one sync-wait per Drain
    (CTRL_NO_STRUCT); Tile's kernel-tail drain carries one wait per live
    semaphore. Split the waits across a chain of drains, one wait each."""
    if getattr(tile.TileContext, "_drain_split_patched", False):
        return

    def _patched(self, tick_clock, wait_clock):
        nc = self.nc
        probe = nc.sync.drain()
        wait_clock.add_sem_waits(
            probe.ins, ScopedClock({None: tick_clock.global_clock}))
        nc.all_engine_barrier()
        popped = nc._tile_sem_poison_stack.pop()
        assert popped is self._sem_poison
        # all engines are quiesced by the barrier above; the sem clear runs
        # on one engine and nothing executes after it, so the second
        # barrier the stock drain emits (~3-4us of split sem-waits under
        # this container's one-wait-per-instruction codegen) is dropped.
        nc.clear_and_free_semaphores(list(self.sems.allocated().values()))

    tile.TileContext._drain_and_barrier = _patched
    tile.TileContext._drain_split_patched = True


_patch_tile_drain()


def _split_json_waits(j, max_waits=1):
    """This walrus codegen accepts at most one sync-wait per instruction.
    Hoist extra waits onto wait-only EventSemaphore instructions inserted
    just before the offender on the same engine queue."""
    n = 0
    for fn in j["functions"]:
        for blk in fn["blocks"]:
            out = []
            for inst in blk["instructions"]:
                si = inst.get("sync_info")
                waits = (si or {}).get("on_wait") or []
                if len(waits) > max_waits:
                    for k, w in enumerate(waits[:-max_waits]):
                        out.append({
                            "debug": inst.get("debug", 0),
                            "engine": inst["engine"],
                            "ins": [], "outs": [],
                            "name": f"{inst['name']}_wsplit{k}",
                            "opcode": "EventSemaphore",
                            "sync_info": {"on_update": [], "on_wait": [w]},
                        })
                        n += 1
                    si["on_wait"] = waits[-max_waits:]
                out.append(inst)
            blk["instructions"] = out
    return n


def _install_wait_splitter(nc):
    import json as _json

    orig = nc.to_json_bytes

    def patched():
        j = _json.loads(orig())
        _split_json_waits(j)
        return _json.dumps(j).encode()

    nc.to_json_bytes = patched

B = 1024
L = 50
D = 128
NV = 8
NH = 16
NU = 100000
NI = 500000
NCORES = 8
BLOC = B // NCORES          # 128 batch rows per core
TP = 64                     # t' pitch in ET
NEG = -1.0e30
FC_IN = NV * D + NH * L     # 1824
NOUT = 2 * D                # 256

LDW_REUSE = bool(int(os.environ.get("BASS_LDW_REUSE", "0")))

# Height-chunk table: heights [8u, 8u+nh) packed as m2 = 16*(i-8u)+j.
# ndt taps accumulate in PSUM; Nt is the t-window (valid-t of the chunk's
# shortest filter); Nb batch rows per matmul so that Nb*Nt <= 512.
CHUNKS = []
_base = 0
for _u in range(7):
    _i0 = 8 * _u
    _nh = min(8, L - _i0)
    _ndt = min(_i0 + 8, L)
    _nt = L - _i0
    _nb = min(BLOC, 512 // _nt)
    _nblk = math.ceil(BLOC / _nb)
    CHUNKS.append(dict(i0=_i0, nh=_nh, ndt=_ndt, nt=_nt, nb=_nb,
                       nblk=_nblk, base=_base))
    _base += _ndt
NWTILES = _base             # 218 weight tiles of [d=128, m2=128]

# conv matmuls for these (u, blk) chase the gather stream; each uses one
# cpsum bank for the whole gather window.
PHASE_A = [(6, 0), (5, 0), (5, 1), (5, 2), (4, 0), (4, 1)]

_NC_CACHE = None

# Set BASS_KERNEL_TRACE=1 to profile; exec time lands in LAST_RESULTS.
LAST_RESULTS = None


def _build_nc():
    f32 = mybir.dt.float32
    bf16 = mybir.dt.bfloat16
    i32 = mybir.dt.int32
    X = mybir.AxisListType.X

    nc = bass.Bass()
    seq_t = nc.dram_tensor("seq_idx", [BLOC, L], i32, kind="ExternalInput")
    uid_t = nc.dram_tensor("uid_idx", [BLOC, 1], i32, kind="ExternalInput")
    item_t = nc.dram_tensor("item_emb", [NI, D], bf16, kind="ExternalInput")
    user_t = nc.dram_tensor("user_emb", [NU, D], f32, kind="ExternalInput")
    whp_t = nc.dram_tensor("whp", [D, NWTILES * 128], bf16, kind="ExternalInput")
    g_t = nc.dram_tensor("g", [D, L * D], bf16, kind="ExternalInput")
    fcwh_t = nc.dram_tensor("fcwh", [128, 7 * D], bf16, kind="ExternalInput")
    masks_t = nc.dram_tensor("masks", [128, 7 * 512], bf16, kind="ExternalInput")
    bh_t = nc.dram_tensor("bh_p", [128, 7], f32, kind="ExternalInput")
    fcb_t = nc.dram_tensor("fcb", [1, D], f32, kind="ExternalInput")
    out_t = nc.dram_tensor("out", [BLOC, NOUT], f32, kind="ExternalOutput")

    with ExitStack() as ctx:
        tc = ctx.enter_context(tile.TileContext(nc))
        const = ctx.enter_context(tc.tile_pool(name="const", bufs=1))
        egath = ctx.enter_context(tc.tile_pool(name="egath", bufs=8))
        etp = ctx.enter_context(tc.tile_pool(name="etp", bufs=1))
        wpool = ctx.enter_context(tc.tile_pool(name="wpool", bufs=1))
        ohp = ctx.enter_context(tc.tile_pool(name="ohp", bufs=1))
        misc = ctx.enter_context(tc.tile_pool(name="misc", bufs=1))
        tpsum = ctx.enter_context(tc.tile_pool(name="tpsum", bufs=1, space="PSUM"))
        cpsum = ctx.enter_context(tc.tile_pool(name="cpsum", bufs=6, space="PSUM"))
        zpsum = ctx.enter_context(tc.tile_pool(name="zpsum", bufs=1, space="PSUM"))

        # --- small constants; seq first (it gates the gathers) ---
        seq_sb = const.tile([BLOC, L], i32)
        nc.sync.dma_start(out=seq_sb[:], in_=seq_t[:])
        uid_sb = const.tile([BLOC, 1], i32)
        nc.sync.dma_start(out=uid_sb[:], in_=uid_t[:])

        # identity must precede the gathers on the gpsimd queue (transposes
        # need it almost immediately; the gather stream occupies the queue
        # for ~55us)
        ident = const.tile([128, 128], bf16)
        make_identity(nc, ident[:])

        # --- per-position indirect gathers (multi-offset DGE is broken on
        # this hw; k=1 is the proven shape). ~1.1us of serial SWDGE each,
        # hidden behind the conv chase. ---
        eg_tiles = []
        for t in range(L):
            eg = egath.tile([BLOC, D], bf16, tag="eg", name=f"eg{t}")
            nc.gpsimd.indirect_dma_start(
                out=eg[:], out_offset=None, in_=item_t[:],
                in_offset=IndirectOffsetOnAxis(ap=seq_sb[:, t:t + 1], axis=0))
            eg_tiles.append(eg)
        pu_sb = misc.tile([BLOC, D], f32, tag="pu")
        nc.gpsimd.indirect_dma_start(
            out=pu_sb[:], out_offset=None, in_=user_t[:],
            in_offset=IndirectOffsetOnAxis(ap=uid_sb[:, :1], axis=0))

        fcb_sb = const.tile([1, D], f32)
        nc.sync.dma_start(out=fcb_sb[:], in_=fcb_t[:])
        bh_sb = const.tile([128, 7], f32)
        nc.sync.dma_start(out=bh_sb[:], in_=bh_t[:])
        ones_sb = const.tile([1, BLOC], f32)
        nc.vector.memset(ones_sb[:], 1.0)

        # --- weight prefetch: everything issued up front, split into
        # pieces, spread over the sync + scalar queues in need order so no
        # conv phase ever waits on a weight transfer ---
        wu_sb = {}
        for u, ch in enumerate(CHUNKS):
            wu_sb[u] = wpool.tile([D, ch["ndt"] * 128], bf16, tag=f"w{u}",
                                  name=f"wu{u}")
        g_sb = const.tile([D, L * D], bf16)
        mask_sb = const.tile([128, 7 * 512], bf16)
        fcwh_sb = const.tile([128, 7 * D], bf16)

        def wu_piece(eng, u, d0, d1):
            ch = CHUNKS[u]
            d1 = min(d1, ch["ndt"])
            eng.dma_start(
                out=wu_sb[u][:, d0 * 128:d1 * 128],
                in_=whp_t[:, (ch["base"] + d0) * 128:(ch["base"] + d1) * 128])

        # need order: u6/u5 taps stream with the gathers; u4 from ~15us;
        # masks at ~55us; then u3/u2/u1/u0 paced by the main phase.
        wu_piece(nc.sync, 6, 0, 16)
        wu_piece(nc.scalar, 5, 0, 16)
        wu_piece(nc.sync, 6, 16, 50)
        wu_piece(nc.scalar, 5, 16, 48)
        nc.sync.dma_start(out=g_sb[:, :25 * D], in_=g_t[:, :25 * D])
        nc.scalar.dma_start(out=g_sb[:, 25 * D:], in_=g_t[:, 25 * D:])
        wu_piece(nc.sync, 4, 0, 20)
        wu_piece(nc.scalar, 4, 20, 40)
        nc.sync.dma_start(out=mask_sb[:], in_=masks_t[:])
        nc.scalar.dma_start(out=fcwh_sb[:], in_=fcwh_t[:])
        wu_piece(nc.sync, 3, 0, 16)
        wu_piece(nc.scalar, 3, 16, 32)
        wu_piece(nc.sync, 2, 0, 24)
        wu_piece(nc.scalar, 1, 0, 16)
        wu_piece(nc.sync, 0, 0, 8)

        # --- ET[d, t', b] in bf16: t-major with b contiguous, so conv
        # matmuls stream unit-stride columns in t-major psum order ---
        et = etp.tile([D, TP, BLOC], bf16)

        # --- fc accumulation PSUM [b, k]; opened by the bias matmul inside
        # the chase (after the first transposes), closed by the last fc
        # matmul. ---
        zp = zpsum.tile([BLOC, D], f32)

        # 4 transpose slots in one PSUM bank; pair-copies drain them.
        tp = tpsum.tile([128, 4, 128], bf16)

        # --- conv bookkeeping ---
        psum_tiles = {}
        fc_pending = []
        oh_tiles = {}
        blocks_left = [ch["nblk"] for ch in CHUNKS]
        n_fc_left = [7]

        def get_ohu(u):
            if u not in oh_tiles:
                oh_tiles[u] = ohp.tile([128, BLOC], bf16, tag=f"oh{u}",
                                       name=f"oh{u}")
            return oh_tiles[u]

        def flush_fc(all_=False):
            keep = 0 if all_ else 1
            while len(fc_pending) > keep:
                uu = fc_pending.pop(0)
                n_fc_left[0] -= 1
                nc.tensor.matmul(out=zp[:], lhsT=oh_tiles[uu][:],
                                 rhs=fcwh_sb[:, uu * D:(uu + 1) * D],
                                 start=False, stop=(n_fc_left[0] == 0))

        def emit_conv(u, blk, dt, reuse, red_eng=None):
            # conv PSUM layout is t-major (col = t*nbb + b) so a trimmed
            # tap's window is a contiguous 2D prefix of the bank.
            ch = CHUNKS[u]
            nt, nb, ndt = ch["nt"], ch["nb"], ch["ndt"]
            b0 = blk * nb
            nbb = min(nb, BLOC - b0)
            n = nbb * nt
            w = min(nt, L - dt)
            key = (u, blk)
            if key not in psum_tiles:
                psum_tiles[key] = cpsum.tile([128, 512], f32, tag="cps",
                                             name=f"cps_{u}_{blk}")
            ps = psum_tiles[key]
            rhs = et[:, dt:dt + w, b0:b0 + nbb]
            mm = nc.tensor.matmul(
                out=ps[:, :nbb * w],
                lhsT=wu_sb[u][:, dt * 128:(dt + 1) * 128],
                rhs=rhs,
                start=(dt == 0), stop=(dt == ndt - 1),
                skip_group_check=True)
            if reuse and LDW_REUSE:
                mm.ins.ldweights = False
            if dt == ndt - 1:
                # only heights r>0 of the chunk have invalid trailing t
                # positions (at most the last 7 columns) -> mask just those
                ps3 = ps[:, :n].rearrange("p (t b) -> p t b", b=nbb)
                m0 = max(0, nt - 7)
                pst = ps[:, m0 * nbb:n].rearrange("p (t b) -> p t b", b=nbb)
                m3 = mask_sb[:, u * 512 + m0 * nb:u * 512 + nt * nb].rearrange(
                    "p (t b) -> p t b", b=nb)[:, :, :nbb]
                nc.vector.tensor_tensor(
                    out=pst, in0=pst, in1=m3, op=mybir.AluOpType.add)
                nc.vector.reduce_max(
                    out=get_ohu(u)[:, b0:b0 + nbb],
                    in_=ps3.rearrange("p t b -> p b t"),
                    axis=X)
                del psum_tiles[key]
                blocks_left[u] -= 1
                if blocks_left[u] == 0:
                    ohu = get_ohu(u)
                    nc.scalar.activation(ohu[:], ohu[:],
                                         mybir.ActivationFunctionType.Relu,
                                         bias=bh_sb[:, u:u + 1])
                    fc_pending.append(u)

        # chase taps: (u, dt) ready once ET cols [dt, dt+w) are copied;
        # copies land in pairs at odd t, so key on the covering odd column.
        chase = {}
        chase_us = sorted({u for (u, _) in PHASE_A})
        for u in chase_us:
            nt = CHUNKS[u]["nt"]
            for dt in range(CHUNKS[u]["ndt"]):
                w = min(nt, L - dt)
                c = dt + w - 1
                c += (c + 1) % 2
                chase.setdefault(min(c, L - 1), []).append((u, dt))

        # --- gather-chase: per position, transpose -> (odd t) pair copy.
        # The G matmuls + conv taps of a pair are deferred one pair so the
        # PE never idles waiting on the copy it just triggered. ---
        def emit_pair_work(c):
            if c < 1:
                return
            for tt in (c - 1, c):
                nc.tensor.matmul(out=zp[:], lhsT=et[:, tt, :],
                                 rhs=g_sb[:, tt * D:(tt + 1) * D],
                                 start=False, stop=False)
            for (u, dt) in chase.get(c, ()):
                blks = [blk for (uu, blk) in PHASE_A if uu == u]
                for j, blk in enumerate(blks):
                    emit_conv(u, blk, dt, reuse=(j > 0))

        for t in range(L):
            s = t % 4
            nc.tensor.transpose(out=tp[:, s, :], in_=eg_tiles[t][:],
                                identity=ident[:])
            if t % 2 == 0:
                continue
            nc.vector.tensor_copy(out=et[:, t - 1:t + 1, :],
                                  in_=tp[:, s - 1:s + 1, :])
            if t == 1:
                nc.tensor.matmul(out=zp[:], lhsT=ones_sb[:], rhs=fcb_sb[:],
                                 start=True, stop=False)
            emit_pair_work(t - 2)
        emit_pair_work(L - 1)

        # --- main phase: remaining blocks, taps outermost within small
        # sets so stationary weights are reused across the set ---
        done_a = set(PHASE_A)
        for u in [4, 3, 2, 1, 0]:
            ch = CHUNKS[u]
            rem = [blk for blk in range(ch["nblk"]) if (u, blk) not in done_a]
            sets = [rem[i:i + 3] for i in range(0, len(rem), 3)]
            for set3 in sets:
                flush_fc()
                for dt in range(ch["ndt"]):
                    for j, blk in enumerate(set3):
                        emit_conv(u, blk, dt, reuse=(j > 0))

        # --- remaining o_h fc matmuls close the zp group ---
        flush_fc(all_=True)

        z_sb = misc.tile([BLOC, D], f32, tag="z")
        nc.scalar.activation(z_sb[:], zp[:], mybir.ActivationFunctionType.Relu)
        nc.sync.dma_start(out=out_t[:, 0:D], in_=z_sb[:])
        nc.sync.dma_start(out=out_t[:, D:NOUT], in_=pu_sb[:])

    return nc


def _to_bf16(x):
    return np.ascontiguousarray(np.asarray(x, np.float32)).astype(
        ml_dtypes.bfloat16)


def _prep_common(user_emb, item_emb, Wv, bv, Wh, bh, fc_W, fc_b):
    f = np.float32
    item_emb = _to_bf16(item_emb)
    user_emb = np.ascontiguousarray(np.asarray(user_emb, f))
    Wh = np.asarray(Wh, f)          # [L, NH, L, D], zero for dt > i
    Wv = np.asarray(Wv, f)          # [NV, L]
    bv = np.asarray(bv, f)
    bh = np.asarray(bh, f)          # [L, NH]
    fc_W = np.asarray(fc_W, f)      # [FC_IN, D]
    fc_b = np.asarray(fc_b, f)

    whp = np.zeros((D, NWTILES * 128), f)
    masks = np.full((128, 7 * 512), 0.0, f)
    fcwh = np.zeros((128, 7 * D), f)
    bh_p = np.zeros((128, 7), f)
    fcw_h = fc_W[NV * D:]           # [800, D]
    for u, ch in enumerate(CHUNKS):
        i0, nh, ndt, nt, nb = ch["i0"], ch["nh"], ch["ndt"], ch["nt"], ch["nb"]
        base = ch["base"]
        wu = Wh[i0:i0 + nh]         # [nh, NH, L, D]
        for dt in range(ndt):
            blkw = wu[:, :, dt, :].reshape(nh * NH, D)
            whp[:, (base + dt) * 128:(base + dt) * 128 + nh * NH] = blkw.T
        m = np.full((128, nb * nt), NEG, f)
        for mm in range(nh * NH):
            i = i0 + mm // NH
            vt = min(L - i, nt)
            row = np.full((nt,), NEG, f)
            row[:vt] = 0.0
            m[mm] = np.repeat(row, nb)   # t-major: col = t*nb + b
        masks[:, u * 512:u * 512 + nb * nt] = m
        fcwh[:nh * NH, u * D:(u + 1) * D] = fcw_h[u * 128:u * 128 + nh * NH]
        bh_p[:nh * NH, u] = bh[i0:i0 + nh].reshape(nh * NH)

    fcv = fc_W[:NV * D].reshape(NV, D, D)
    g = np.einsum("vt,vdk->tdk", Wv, fcv)            # [L, D, D]
    g = np.ascontiguousarray(g.transpose(1, 0, 2).reshape(D, L * D))
    fcb = (fc_b + np.einsum("v,vdk->k", bv, fcv)).reshape(1, D).astype(f)

    return dict(item_emb=item_emb, user_emb=user_emb, whp=_to_bf16(whp),
                g=_to_bf16(g), fcwh=_to_bf16(fcwh), masks=_to_bf16(masks),
                bh_p=bh_p, fcb=fcb)


def make_in_maps(user_ids, item_seq, user_emb, item_emb, Wv, bv, Wh, bh,
                 fc_W, fc_b):
    common = _prep_common(user_emb, item_emb, Wv, bv, Wh, bh, fc_W, fc_b)
    user_ids = np.asarray(user_ids).astype(np.int32).reshape(B, 1)
    item_seq = np.asarray(item_seq).astype(np.int32).reshape(B, L)
    in_maps = []
    for c in range(NCORES):
        m = dict(common)
        m["seq_idx"] = np.ascontiguousarray(item_seq[c * BLOC:(c + 1) * BLOC])
        m["uid_idx"] = np.ascontiguousarray(user_ids[c * BLOC:(c + 1) * BLOC])
        in_maps.append(m)
    return in_maps


def get_nc():
    global _NC_CACHE
    if _NC_CACHE is None:
        _NC_CACHE = _build_nc()
        _install_wait_splitter(_NC_CACHE)
    return _NC_CACHE


def kernel(**inputs) -> np.ndarray:
    global LAST_RESULTS
    in_maps = make_in_maps(**inputs)
    nc = get_nc()
    trace = bool(int(os.environ.get("BASS_KERNEL_TRACE", "0")))
    res = run_bass_kernel_spmd(nc, in_maps, list(range(NCORES)), trace=trace)
    LAST_RESULTS = res
    return np.concatenate([res.results[c]["out"] for c in range(NCORES)], axis=0)


# revision 30
# speedup vs baseline: 1.0075x; 1.0019x over previous
"""Trainium2 Bass kernel for nn_CaserQueryEncoder.

Model (B=1024, L=50, D=128, NV=8, NH=16):
  P_u = user_emb[user_ids]                                   [B, D]
  E   = item_emb[item_seq]                                   [B, L, D]
  o_v = einsum('btd,vt->bvd', E, Wv) + bv                    [B, NV*D]
  conv[b,i,j,t] = sum_{dt<=i} <E[b, t+dt, :], Wh[i,j,dt,:]>  (Wh zero for dt>i)
  o_h[b,i,j] = max over valid t (t <= 49-i) of relu(conv + bh)
  z = relu([o_v, o_h] @ fc_W + fc_b)                         [B, D]
  out = [z, P_u]                                             [B, 2D]

Strategy: pure data parallel, 128 batch rows per core x 8 cores, no
collectives. Per core:
  - item embeddings stored bf16 in DRAM; looked-up rows arrive via batched
    indirect DMAs (7 groups of ~8 seq positions -> ~1k descriptors each,
    amortizing the ~1us fixed SWDGE cost that a per-position gather pays
    50x). PE transposes (bf16, via identity) build ET[d, b, t'].
  - horizontal conv in bf16 as PSUM-accumulated shifted matmuls: heights
    packed 8 per chunk (x16 filters = 128 weight columns); for each tap dt
    the rhs is ET shifted by dt; PSUM accumulates over taps. Per-tap
    t-windows are trimmed to min(nt, 50-dt) (later columns only read the
    zero pad). Invalid (height, t) cells get an additive -1e30 mask before
    the max-reduce; max(relu(x+b)) == relu(max(x)+b) so relu+bias follow
    the reduce.
  - blocks are processed in small sets with taps outermost so consecutive
    matmuls share stationary weights; those continuation matmuls set
    ldweights=False to skip the per-matmul weight reload.
  - vertical conv never materialized: G[t,d,k] = sum_v Wv[v,t]*fc_W[v*128+d,k]
    is precomputed on host (bf16) and ET_t @ G_t is accumulated straight
    into the fc PSUM.
  - fc bias added via a K=1 ones-matmul; relu on the scalar engine.
"""

import math
import os
import sys
from contextlib import ExitStack

import numpy as np
import ml_dtypes

sys.path.insert(0, "/opt/trn_rl_repo")

import concourse.bass as bass
import concourse.tile as tile
from concourse import mybir
from concourse.bass import IndirectOffsetOnAxis
from concourse.bass_utils import run_bass_kernel_spmd
from concourse.masks import make_identity
from concourse.vector_clock import ScopedClock


def _patch_tile_drain():
    """This container's walrus codegen only accepts one sync-wait per Drain
    (CTRL_NO_STRUCT); Tile's kernel-tail drain carries one wait per live
    semaphore. Split the waits across a chain of drains, one wait each."""
    if getattr(tile.TileContext, "_drain_split_patched", False):
        return

    def _patched(self, tick_clock, wait_clock):
        nc = self.nc
        probe = nc.sync.drain()
        wait_clock.add_sem_waits(
            probe.ins, ScopedClock({None: tick_clock.global_clock}))
        # sem-only barrier: same retirement guarantee as the full barrier
        # (each engine's inc is its last instruction) at a fraction of the
        # serialized split-wait cost under this one-wait-per-inst codegen.
        nc.all_engine_barrier(sem_only=True)
        popped = nc._tile_sem_poison_stack.pop()
        assert popped is self._sem_poison
        # all engines are quiesced by the barrier above; the sem clear runs
        # on one engine and nothing executes after it, so the second
        # barrier the stock drain emits (~3-4us of split sem-waits under
        # this container's one-wait-per-instruction codegen) is dropped.
        nc.clear_and_free_semaphores(list(self.sems.allocated().values()))

    tile.TileContext._drain_and_barrier = _patched
    tile.TileContext._drain_split_patched = True


_patch_tile_drain()


def _split_json_waits(j, max_waits=1):
    """This walrus codegen accepts at most one sync-wait per instruction.
    Hoist extra waits onto wait-only EventSemaphore instructions inserted
    just before the offender on the same engine queue."""
    n = 0
    for fn in j["functions"]:
        for blk in fn["blocks"]:
            out = []
            for inst in blk["instructions"]:
                si = inst.get("sync_info")
                waits = (si or {}).get("on_wait") or []
                if len(waits) > max_waits:
                    for k, w in enumerate(waits[:-max_waits]):
                        out.append({
                            "debug": inst.get("debug", 0),
                            "engine": inst["engine"],
                            "ins": [], "outs": [],
                            "name": f"{inst['name']}_wsplit{k}",
                            "opcode": "EventSemaphore",
                            "sync_info": {"on_update": [], "on_wait": [w]},
                        })
                        n += 1
                    si["on_wait"] = waits[-max_waits:]
                out.append(inst)
            blk["instructions"] = out
    return n


def _install_wait_splitter(nc):
    import json as _json

    orig = nc.to_json_bytes

    def patched():
        j = _json.loads(orig())
        _split_json_waits(j)
        return _json.dumps(j).encode()

    nc.to_json_bytes = patched

B = 1024
L = 50
D = 128
NV = 8
NH = 16
NU = 100000
NI = 500000
NCORES = 8
BLOC = B // NCORES          # 128 batch rows per core
TP = 64                     # t' pitch in ET
NEG = -1.0e30
FC_IN = NV * D + NH * L     # 1824
NOUT = 2 * D                # 256

LDW_REUSE = bool(int(os.environ.get("BASS_LDW_REUSE", "0")))

# Height-chunk table: heights [8u, 8u+nh) packed as m2 = 16*(i-8u)+j.
# ndt taps accumulate in PSUM; Nt is the t-window (valid-t of the chunk's
# shortest filter); Nb batch rows per matmul so that Nb*Nt <= 512.
CHUNKS = []
_base = 0
for _u in range(7):
    _i0 = 8 * _u
    _nh = min(8, L - _i0)
    _ndt = min(_i0 + 8, L)
    _nt = L - _i0
    _nb = min(BLOC, 512 // _nt)
    _nblk = math.ceil(BLOC / _nb)
    CHUNKS.append(dict(i0=_i0, nh=_nh, ndt=_ndt, nt=_nt, nb=_nb,
                       nblk=_nblk, base=_base))
    _base += _ndt
NWTILES = _base             # 218 weight tiles of [d=128, m2=128]

# conv matmuls for these (u, blk) chase the gather stream; each uses one
# cpsum bank for the whole gather window.
PHASE_A = [(6, 0), (5, 0), (5, 1), (5, 2), (4, 0), (4, 1)]

_NC_CACHE = None

# Set BASS_KERNEL_TRACE=1 to profile; exec time lands in LAST_RESULTS.
LAST_RESULTS = None


def _build_nc():
    f32 = mybir.dt.float32
    bf16 = mybir.dt.bfloat16
    i32 = mybir.dt.int32
    X = mybir.AxisListType.X

    nc = bass.Bass()
    seq_t = nc.dram_tensor("seq_idx", [BLOC, L], i32, kind="ExternalInput")
    uid_t = nc.dram_tensor("uid_idx", [BLOC, 1], i32, kind="ExternalInput")
    item_t = nc.dram_tensor("item_emb", [NI, D], bf16, kind="ExternalInput")
    user_t = nc.dram_tensor("user_emb", [NU, D], f32, kind="ExternalInput")
    whp_t = nc.dram_tensor("whp", [D, NWTILES * 128], bf16, kind="ExternalInput")
    g_t = nc.dram_tensor("g", [D, L * D], bf16, kind="ExternalInput")
    fcwh_t = nc.dram_tensor("fcwh", [128, 7 * D], bf16, kind="ExternalInput")
    masks_t = nc.dram_tensor("masks", [128, 7 * 512], bf16, kind="ExternalInput")
    bh_t = nc.dram_tensor("bh_p", [128, 7], f32, kind="ExternalInput")
    fcb_t = nc.dram_tensor("fcb", [1, D], f32, kind="ExternalInput")
    out_t = nc.dram_tensor("out", [BLOC, NOUT], f32, kind="ExternalOutput")

    with ExitStack() as ctx:
        tc = ctx.enter_context(tile.TileContext(nc))
        const = ctx.enter_context(tc.tile_pool(name="const", bufs=1))
        egath = ctx.enter_context(tc.tile_pool(name="egath", bufs=8))
        etp = ctx.enter_context(tc.tile_pool(name="etp", bufs=1))
        wpool = ctx.enter_context(tc.tile_pool(name="wpool", bufs=1))
        ohp = ctx.enter_context(tc.tile_pool(name="ohp", bufs=1))
        misc = ctx.enter_context(tc.tile_pool(name="misc", bufs=1))
        tpsum = ctx.enter_context(tc.tile_pool(name="tpsum", bufs=1, space="PSUM"))
        cpsum = ctx.enter_context(tc.tile_pool(name="cpsum", bufs=6, space="PSUM"))
        zpsum = ctx.enter_context(tc.tile_pool(name="zpsum", bufs=1, space="PSUM"))

        # --- small constants; seq first (it gates the gathers) ---
        seq_sb = const.tile([BLOC, L], i32)
        nc.sync.dma_start(out=seq_sb[:], in_=seq_t[:])
        uid_sb = const.tile([BLOC, 1], i32)
        nc.sync.dma_start(out=uid_sb[:], in_=uid_t[:])

        # identity must precede the gathers on the gpsimd queue (transposes
        # need it almost immediately; the gather stream occupies the queue
        # for ~55us)
        ident = const.tile([128, 128], bf16)
        make_identity(nc, ident[:])

        # --- per-position indirect gathers (multi-offset DGE is broken on
        # this hw; k=1 is the proven shape). ~1.1us of serial SWDGE each,
        # hidden behind the conv chase. ---
        eg_tiles = []
        for t in range(L):
            eg = egath.tile([BLOC, D], bf16, tag="eg", name=f"eg{t}")
            nc.gpsimd.indirect_dma_start(
                out=eg[:], out_offset=None, in_=item_t[:],
                in_offset=IndirectOffsetOnAxis(ap=seq_sb[:, t:t + 1], axis=0))
            eg_tiles.append(eg)
        pu_sb = misc.tile([BLOC, D], f32, tag="pu")
        nc.gpsimd.indirect_dma_start(
            out=pu_sb[:], out_offset=None, in_=user_t[:],
            in_offset=IndirectOffsetOnAxis(ap=uid_sb[:, :1], axis=0))

        fcb_sb = const.tile([1, D], f32)
        nc.sync.dma_start(out=fcb_sb[:], in_=fcb_t[:])
        bh_sb = const.tile([128, 7], f32)
        nc.sync.dma_start(out=bh_sb[:], in_=bh_t[:])
        ones_sb = const.tile([1, BLOC], f32)
        nc.vector.memset(ones_sb[:], 1.0)

        # --- weight prefetch: everything issued up front, split into
        # pieces, spread over the sync + scalar queues in need order so no
        # conv phase ever waits on a weight transfer ---
        wu_sb = {}
        for u, ch in enumerate(CHUNKS):
            wu_sb[u] = wpool.tile([D, ch["ndt"] * 128], bf16, tag=f"w{u}",
                                  name=f"wu{u}")
        g_sb = const.tile([D, L * D], bf16)
        mask_sb = const.tile([128, 7 * 512], bf16)
        fcwh_sb = const.tile([128, 7 * D], bf16)

        def wu_piece(eng, u, d0, d1):
            ch = CHUNKS[u]
            d1 = min(d1, ch["ndt"])
            eng.dma_start(
                out=wu_sb[u][:, d0 * 128:d1 * 128],
                in_=whp_t[:, (ch["base"] + d0) * 128:(ch["base"] + d1) * 128])

        # need order: u6/u5 taps stream with the gathers; u4 from ~15us;
        # masks at ~55us; then u3/u2/u1/u0 paced by the main phase.
        wu_piece(nc.sync, 6, 0, 16)
        wu_piece(nc.scalar, 5, 0, 16)
        wu_piece(nc.sync, 6, 16, 50)
        wu_piece(nc.scalar, 5, 16, 48)
        nc.sync.dma_start(out=g_sb[:, :25 * D], in_=g_t[:, :25 * D])
        nc.scalar.dma_start(out=g_sb[:, 25 * D:], in_=g_t[:, 25 * D:])
        wu_piece(nc.sync, 4, 0, 20)
        wu_piece(nc.scalar, 4, 20, 40)
        nc.sync.dma_start(out=mask_sb[:], in_=masks_t[:])
        nc.scalar.dma_start(out=fcwh_sb[:], in_=fcwh_t[:])
        wu_piece(nc.sync, 3, 0, 16)
        wu_piece(nc.scalar, 3, 16, 32)
        wu_piece(nc.sync, 2, 0, 24)
        wu_piece(nc.scalar, 1, 0, 16)
        wu_piece(nc.sync, 0, 0, 8)

        # --- ET[d, t', b] in bf16: t-major with b contiguous, so conv
        # matmuls stream unit-stride columns in t-major psum order ---
        et = etp.tile([D, TP, BLOC], bf16)

        # --- fc accumulation PSUM [b, k]; opened by the bias matmul inside
        # the chase (after the first transposes), closed by the last fc
        # matmul. ---
        zp = zpsum.tile([BLOC, D], f32)

        # 4 transpose slots in one PSUM bank; pair-copies drain them.
        tp = tpsum.tile([128, 4, 128], bf16)

        # --- conv bookkeeping ---
        psum_tiles = {}
        fc_pending = []
        oh_tiles = {}
        blocks_left = [ch["nblk"] for ch in CHUNKS]
        n_fc_left = [7]

        def get_ohu(u):
            if u not in oh_tiles:
                oh_tiles[u] = ohp.tile([128, BLOC], bf16, tag=f"oh{u}",
                                       name=f"oh{u}")
            return oh_tiles[u]

        def flush_fc(all_=False):
            keep = 0 if all_ else 1
            while len(fc_pending) > keep:
                uu = fc_pending.pop(0)
                n_fc_left[0] -= 1
                nc.tensor.matmul(out=zp[:], lhsT=oh_tiles[uu][:],
                                 rhs=fcwh_sb[:, uu * D:(uu + 1) * D],
                                 start=False, stop=(n_fc_left[0] == 0))

        def emit_conv(u, blk, dt, reuse, red_eng=None):
            # conv PSUM layout is t-major (col = t*nbb + b) so a trimmed
            # tap's window is a contiguous 2D prefix of the bank.
            ch = CHUNKS[u]
            nt, nb, ndt = ch["nt"], ch["nb"], ch["ndt"]
            b0 = blk * nb
            nbb = min(nb, BLOC - b0)
            n = nbb * nt
            w = min(nt, L - dt)
            key = (u, blk)
            if key not in psum_tiles:
                psum_tiles[key] = cpsum.tile([128, 512], f32, tag="cps",
                                             name=f"cps_{u}_{blk}")
            ps = psum_tiles[key]
            rhs = et[:, dt:dt + w, b0:b0 + nbb]
            mm = nc.tensor.matmul(
                out=ps[:, :nbb * w],
                lhsT=wu_sb[u][:, dt * 128:(dt + 1) * 128],
                rhs=rhs,
                start=(dt == 0), stop=(dt == ndt - 1),
                skip_group_check=True)
            if reuse and LDW_REUSE:
                mm.ins.ldweights = False
            if dt == ndt - 1:
                # only heights r>0 of the chunk have invalid trailing t
                # positions (at most the last 7 columns) -> mask just those
                ps3 = ps[:, :n].rearrange("p (t b) -> p t b", b=nbb)
                m0 = max(0, nt - 7)
                pst = ps[:, m0 * nbb:n].rearrange("p (t b) -> p t b", b=nbb)
                m3 = mask_sb[:, u * 512 + m0 * nb:u * 512 + nt * nb].rearrange(
                    "p (t b) -> p t b", b=nb)[:, :, :nbb]
                nc.vector.tensor_tensor(
                    out=pst, in0=pst, in1=m3, op=mybir.AluOpType.add)
                nc.vector.reduce_max(
                    out=get_ohu(u)[:, b0:b0 + nbb],
                    in_=ps3.rearrange("p t b -> p b t"),
                    axis=X)
                del psum_tiles[key]
                blocks_left[u] -= 1
                if blocks_left[u] == 0:
                    ohu = get_ohu(u)
                    nc.scalar.activation(ohu[:], ohu[:],
                                         mybir.ActivationFunctionType.Relu,
                                         bias=bh_sb[:, u:u + 1])
                    fc_pending.append(u)

        # chase taps: (u, dt) ready once ET cols [dt, dt+w) are copied;
        # copies land in pairs at odd t, so key on the covering odd column.
        chase = {}
        chase_us = sorted({u for (u, _) in PHASE_A})
        for u in chase_us:
            nt = CHUNKS[u]["nt"]
            for dt in range(CHUNKS[u]["ndt"]):
                w = min(nt, L - dt)
                c = dt + w - 1
                c += (c + 1) % 2
                chase.setdefault(min(c, L - 1), []).append((u, dt))

        # --- gather-chase: per position, transpose -> (odd t) pair copy.
        # The G matmuls + conv taps of a pair are deferred one pair so the
        # PE never idles waiting on the copy it just triggered. ---
        def emit_pair_work(c):
            if c < 1:
                return
            for tt in (c - 1, c):
                nc.tensor.matmul(out=zp[:], lhsT=et[:, tt, :],
                                 rhs=g_sb[:, tt * D:(tt + 1) * D],
                                 start=False, stop=False)
            for (u, dt) in chase.get(c, ()):
                blks = [blk for (uu, blk) in PHASE_A if uu == u]
                for j, blk in enumerate(blks):
                    emit_conv(u, blk, dt, reuse=(j > 0))

        for t in range(L):
            s = t % 4
            nc.tensor.transpose(out=tp[:, s, :], in_=eg_tiles[t][:],
                                identity=ident[:])
            if t % 2 == 0:
                continue
            nc.vector.tensor_copy(out=et[:, t - 1:t + 1, :],
                                  in_=tp[:, s - 1:s + 1, :])
            if t == 1:
                nc.tensor.matmul(out=zp[:], lhsT=ones_sb[:], rhs=fcb_sb[:],
                                 start=True, stop=False)
            emit_pair_work(t - 2)
        emit_pair_work(L - 1)

        # --- main phase: remaining blocks, taps outermost within small
        # sets so stationary weights are reused across the set ---
        done_a = set(PHASE_A)
        for u in [4, 3, 2, 1, 0]:
            ch = CHUNKS[u]
            rem = [blk for blk in range(ch["nblk"]) if (u, blk) not in done_a]
            sets = [rem[i:i + 3] for i in range(0, len(rem), 3)]
            for set3 in sets:
                flush_fc()
                for dt in range(ch["ndt"]):
                    for j, blk in enumerate(set3):
                        emit_conv(u, blk, dt, reuse=(j > 0))

        # --- remaining o_h fc matmuls close the zp group ---
        flush_fc(all_=True)

        z_sb = misc.tile([BLOC, D], f32, tag="z")
        nc.scalar.activation(z_sb[:], zp[:], mybir.ActivationFunctionType.Relu)
        nc.sync.dma_start(out=out_t[:, 0:D], in_=z_sb[:])
        nc.sync.dma_start(out=out_t[:, D:NOUT], in_=pu_sb[:])

    return nc


def _to_bf16(x):
    return np.ascontiguousarray(np.asarray(x, np.float32)).astype(
        ml_dtypes.bfloat16)


def _prep_common(user_emb, item_emb, Wv, bv, Wh, bh, fc_W, fc_b):
    f = np.float32
    item_emb = _to_bf16(item_emb)
    user_emb = np.ascontiguousarray(np.asarray(user_emb, f))
    Wh = np.asarray(Wh, f)          # [L, NH, L, D], zero for dt > i
    Wv = np.asarray(Wv, f)          # [NV, L]
    bv = np.asarray(bv, f)
    bh = np.asarray(bh, f)          # [L, NH]
    fc_W = np.asarray(fc_W, f)      # [FC_IN, D]
    fc_b = np.asarray(fc_b, f)

    whp = np.zeros((D, NWTILES * 128), f)
    masks = np.full((128, 7 * 512), 0.0, f)
    fcwh = np.zeros((128, 7 * D), f)
    bh_p = np.zeros((128, 7), f)
    fcw_h = fc_W[NV * D:]           # [800, D]
    for u, ch in enumerate(CHUNKS):
        i0, nh, ndt, nt, nb = ch["i0"], ch["nh"], ch["ndt"], ch["nt"], ch["nb"]
        base = ch["base"]
        wu = Wh[i0:i0 + nh]         # [nh, NH, L, D]
        for dt in range(ndt):
            blkw = wu[:, :, dt, :].reshape(nh * NH, D)
            whp[:, (base + dt) * 128:(base + dt) * 128 + nh * NH] = blkw.T
        m = np.full((128, nb * nt), NEG, f)
        for mm in range(nh * NH):
            i = i0 + mm // NH
            vt = min(L - i, nt)
            row = np.full((nt,), NEG, f)
            row[:vt] = 0.0
            m[mm] = np.repeat(row, nb)   # t-major: col = t*nb + b
        masks[:, u * 512:u * 512 + nb * nt] = m
        fcwh[:nh * NH, u * D:(u + 1) * D] = fcw_h[u * 128:u * 128 + nh * NH]
        bh_p[:nh * NH, u] = bh[i0:i0 + nh].reshape(nh * NH)

    fcv = fc_W[:NV * D].reshape(NV, D, D)
    g = np.einsum("vt,vdk->tdk", Wv, fcv)            # [L, D, D]
    g = np.ascontiguousarray(g.transpose(1, 0, 2).reshape(D, L * D))
    fcb = (fc_b + np.einsum("v,vdk->k", bv, fcv)).reshape(1, D).astype(f)

    return dict(item_emb=item_emb, user_emb=user_emb, whp=_to_bf16(whp),
                g=_to_bf16(g), fcwh=_to_bf16(fcwh), masks=_to_bf16(masks),
                bh_p=bh_p, fcb=fcb)


def make_in_maps(user_ids, item_seq, user_emb, item_emb, Wv, bv, Wh, bh,
                 fc_W, fc_b):
    common = _prep_common(user_emb, item_emb, Wv, bv, Wh, bh, fc_W, fc_b)
    user_ids = np.asarray(user_ids).astype(np.int32).reshape(B, 1)
    item_seq = np.asarray(item_seq).astype(np.int32).reshape(B, L)
    in_maps = []
    for c in range(NCORES):
        m = dict(common)
        m["seq_idx"] = np.ascontiguousarray(item_seq[c * BLOC:(c + 1) * BLOC])
        m["uid_idx"] = np.ascontiguousarray(user_ids[c * BLOC:(c + 1) * BLOC])
        in_maps.append(m)
    return in_maps


def get_nc():
    global _NC_CACHE
    if _NC_CACHE is None:
        _NC_CACHE = _build_nc()
        _install_wait_splitter(_NC_CACHE)
    return _NC_CACHE


def kernel(**inputs) -> np.ndarray:
    global LAST_RESULTS
    in_maps = make_in_maps(**inputs)
    nc = get_nc()
    trace = bool(int(os.environ.get("BASS_KERNEL_TRACE", "0")))
    res = run_bass_kernel_spmd(nc, in_maps, list(range(NCORES)), trace=trace)
    LAST_RESULTS = res
    return np.concatenate([res.results[c]["out"] for c in range(NCORES)], axis=0)
